# revision 32
# baseline (speedup 1.0000x reference)
"""Trainium2 Bass kernel for LGRL classifier decoder (segment softmax-pool MLP).

Math (reference):
    extra = io_embed.reshape(B, Y)[segment_ids]                # (T, Y)
    h1 = relu([ps_data, extra] @ W1 + b1)
    h2 = relu(h1 @ W2 + b2)
    logits = (h2 @ W3 + b3)[:, 0]
    w = segment_softmax(logits)
    pooled = segment_sum(w * ps_data)                          # (B, X)
    out = relu(pooled @ Wf1 + bf1) @ Wf2 + bf2                 # (B, 2)

Key transformations:
  * Tokens are sharded by SEGMENT BLOCKS: core c owns all tokens of
    segments [8c, 8c+8) (segment_ids are sorted), padded with zero
    tokens to a common tloc.  All segment reductions are core-local --
    no collectives at all.  Core c emits output rows [8c, 8c+8).
  * [ps, extra] @ W1 = ps @ W1a + onehot(seg) @ (io_flat @ W1b + b1):
    seg_contrib = io_flat @ W1b + b1 is precomputed (B,H) on the host;
    on device it enters h1 via a tiny rank-8 one-hot matmul.
  * per-segment max subtraction in the softmax is dropped: softmax is
    shift-invariant and logits are O(1), so exp() is safe in fp32.
    b3 is dropped for the same reason.
  * pooling scales the 8-wide one-hot by e (not the 512-wide ps):
    num = (onehot * e)^T @ ps, den = onehot^T @ e, both on the PE.
  * ps is shipped twice from the host: token-major bf16 (pooling) and
    feature-major fp8 (h1 moving operand) -- no on-device transposes.
  * h1/h2/logit matmuls run in fp8 DoubleRow (~1.5x bf16 rate);
    accumulation is fp32 in PSUM.
"""

import numpy as np
import ml_dtypes

import concourse.bass as bass
import concourse.mybir as mybir
import concourse.tile as tile
from concourse import bacc
from concourse.bass_utils import run_bass_kernel_spmd
from concourse.masks import make_identity

B = 64
T = 65536
X = 512
KIO = 5
Y = X * KIO          # 2560
H = 512
NCORES = 8
BSEG = B // NCORES   # 8 segments per core
P = 128
FP32 = mybir.dt.float32
BF16 = mybir.dt.bfloat16
FP8 = mybir.dt.float8e4
AF = mybir.ActivationFunctionType
ALU = mybir.AluOpType
DR = mybir.MatmulPerfMode.DoubleRow

KC = X // P          # 4 contraction chunks for 512-dims
HC = H // P          # 4 output chunks for 512-dims
MT = 512             # tokens per MLP tile
NSUB = MT // P       # 128-token subtiles per MLP tile
NPRE = 3             # tiles of ps prefetched before the main loop


def build(tloc):
    """Build + compile the SPMD kernel for per-core token count `tloc`."""
    nt = tloc // MT
    nc = bacc.Bacc(
        "TRN2", target_bir_lowering=False, debug=False, num_devices=NCORES
    )

    psm = nc.dram_tensor("psm", [P, nt, NSUB, X], BF16, kind="ExternalInput").ap()
    pst = nc.dram_tensor("pst", [P, nt, KC, MT], FP8, kind="ExternalInput").ap()
    stm = nc.dram_tensor("stm", [P, nt, NSUB, BSEG], BF16, kind="ExternalInput").ap()
    # segst: per 32-row band b: [8, H | tloc] = seg_contrib || one-hot^T
    segst = nc.dram_tensor("segst", [BSEG, H + tloc], BF16, kind="ExternalInput").ap()
    # w8: w1a || w2 || w3 packed fp8 (pair-dim stride 1040 % 16 == 0)
    w8 = nc.dram_tensor("w8", [P, KC, 2 * H + 16], FP8, kind="ExternalInput").ap()
    # wf: wf1 || wf2 || bcol packed bf16
    wf = nc.dram_tensor("wf", [P, KC, H + 10], BF16, kind="ExternalInput").ap()
    # f32: b2c || bf1c || bf2 packed fp32
    f32 = nc.dram_tensor("f32", [P, 9], FP32, kind="ExternalInput").ap()
    outT = nc.dram_tensor("outT", [2, BSEG], FP32, kind="ExternalOutput").ap()

    with tile.TileContext(nc) as tc:
        with (
            tc.tile_pool(name="const", bufs=1) as cpool,
            tc.tile_pool(name="work", bufs=2) as wpool,
            tc.tile_pool(name="psum", bufs=1, space="PSUM") as ppool,
        ):
            # ---------------- constants / weights ----------------
            ident = cpool.tile([P, P], BF16)
            make_identity(nc, ident)
            identf = cpool.tile([1, 1], FP32)
            nc.gpsimd.memset(identf, 1.0)

            def _tile_dma(j):
                ps_bf = wpool.tile([P, NSUB, X], BF16, tag="psm", bufs=4,
                                   name=f"psm_{j}")
                nc.gpsimd.dma_start(ps_bf, psm[:, j])
                psT = wpool.tile([P, KC, MT], FP8, tag="pst", bufs=4,
                                 name=f"pst_{j}")
                nc.sync.dma_start(psT, pst[:, j])
                return ps_bf, psT

            # w1a/w2/w3 in one fp8 blob first on gpsimd (gates tile-0 h1)
            w8_sb = cpool.tile([P, KC, 2 * H + 16], FP8)
            nc.gpsimd.dma_start(w8_sb, w8)
            pre = [_tile_dma(0)]
            # seg/st replicated to the four 32-partition row bands so the
            # four rank-8 seg matmuls run concurrently via tile_position
            segst_sb = cpool.tile([P, H + tloc], BF16)
            for b in range(HC):
                nc.sync.dma_start(
                    segst_sb[32 * b : 32 * b + BSEG, :], segst)
            for j in range(1, NPRE):
                pre.append(_tile_dma(j))
            stm_sb = cpool.tile([P, nt, NSUB, BSEG], BF16)
            nc.gpsimd.dma_start(stm_sb, stm)
            ones_sb = cpool.tile([P, 1], BF16)
            nc.gpsimd.memset(ones_sb, 1.0)
            wf_sb = cpool.tile([P, KC, H + 10], BF16)
            nc.gpsimd.dma_start(wf_sb, wf)
            f32_sb = cpool.tile([P, 9], FP32)
            nc.gpsimd.dma_start(f32_sb, f32)

            # warm the PE clock (HAM) with identity matmuls while the
            # first DMAs land; the result is never read
            warm = ppool.tile([P, MT], FP32, tag="h1h2", bufs=4)
            for _ in range(34):
                nc.tensor.matmul(warm[:, 0:P], ident, ident, start=True, stop=True)

            # ---------------- main loop over MLP tiles ----------------
            # pool accumulates into four 8-partition col bands (one per
            # 128-token subtile) of a single PSUM bank; summed at the end
            pool_psum = ppool.tile([P, H], FP32, tag="pool", bufs=1)
            den_psum = ppool.tile([BSEG, 1], FP32, tag="den", bufs=1)
            prev = None  # (j, ps_bf, e_row) of previous tile

            def emit_echain(jp, p_erow):
                # e-row -> column layout via PE (input must be SBUF), then
                # tiny one-hot scale on the vector engine
                eTp = ppool.tile([P, NSUB], FP32, tag="eT", bufs=1)
                for s in range(NSUB):
                    nc.tensor.transpose(
                        eTp[:, s : s + 1],
                        p_erow[0:1, s * P : (s + 1) * P],
                        identf[0:1, 0:1],
                    )
                e_col = wpool.tile([P, NSUB], FP32, tag="ecol", bufs=2)
                nc.vector.tensor_copy(e_col, eTp)
                stm_e = wpool.tile([P, NSUB, BSEG], BF16, tag="stme", bufs=2)
                for s in range(NSUB):
                    nc.vector.tensor_scalar_mul(
                        stm_e[:, s, :], stm_sb[:, jp, s, :], e_col[:, s : s + 1]
                    )
                return stm_e

            def emit_pool(jp, p_psbf, stm_e):
                # 4 col-banded pool matmuls (M=8 each) run concurrently
                for s in range(NSUB):
                    nc.tensor.matmul(
                        pool_psum[32 * s : 32 * s + BSEG, :],
                        stm_e[:, s, :], p_psbf[:, s, :],
                        start=(jp == 0), stop=(jp == nt - 1),
                        tile_position=(0, 32 * s),
                        skip_group_check=True,
                    )

            def emit_den(jp, stm_e):
                # den = stm_e^T @ ones (exact same bf16 e values as num)
                for s in range(NSUB):
                    sub = jp * NSUB + s
                    nc.tensor.matmul(
                        den_psum, stm_e[:, s, :], ones_sb,
                        start=(sub == 0), stop=(sub == nt * NSUB - 1),
                    )

            for j in range(nt):
                if j < NPRE:
                    ps_bf, psT = pre[j]
                else:
                    ps_bf, psT = _tile_dma(j)

                # h1 = relu(psT-major matmuls + rank-8 seg broadcast).
                # The four rank-8 seg matmuls go first (start=True clears
                # the banks) on distinct PE row bands -> they overlap.
                h1_sb = wpool.tile([P, KC, MT], FP8, tag="h1", bufs=3)
                h1ps = [
                    ppool.tile([P, MT], FP32, tag="h1h2", bufs=4,
                               name=f"h1p_{j}_{hc}")
                    for hc in range(HC)
                ]
                for hc in range(HC):
                    nc.tensor.matmul(
                        h1ps[hc],
                        segst_sb[32 * hc : 32 * hc + BSEG,
                                 hc * P : (hc + 1) * P],
                        segst_sb[32 * hc : 32 * hc + BSEG,
                                 H + j * MT : H + (j + 1) * MT],
                        start=True,
                        stop=False,
                        tile_position=(32 * hc, 0),
                        skip_group_check=True,
                    )
                for hc in range(HC):
                    for kc in range(0, KC, 2):
                        nc.tensor.matmul(
                            h1ps[hc],
                            w8_sb[:, kc : kc + 2, hc * P : (hc + 1) * P],
                            psT[:, kc : kc + 2, :],
                            start=False,
                            stop=(kc == KC - 2),
                            perf_mode=DR,
                            skip_group_check=True,
                        )
                    if hc % 2 == 0:
                        nc.scalar.activation(h1_sb[:, hc, :], h1ps[hc], AF.Relu)
                    else:
                        nc.vector.tensor_scalar_max(h1_sb[:, hc, :], h1ps[hc], 0.0)

                # previous tile's e-transposes + scale (its exp on the
                # scalar engine completed during our h1 matmuls)
                prev_pool = None
                if prev is not None:
                    jp, p_psbf, p_erow = prev
                    stm_e = emit_echain(jp, p_erow)
                    prev_pool = (jp, p_psbf, stm_e)
                    prev = None

                # h2
                h2_sb = wpool.tile([P, KC, MT], FP8, tag="h2", bufs=3)
                for hc in range(HC):
                    h2p = ppool.tile([P, MT], FP32, tag="h1h2", bufs=4)
                    for kc in range(0, KC, 2):
                        nc.tensor.matmul(
                            h2p,
                            w8_sb[:, kc : kc + 2, H + hc * P : H + (hc + 1) * P],
                            h1_sb[:, kc : kc + 2, :],
                            start=(kc == 0),
                            stop=(kc == KC - 2),
                            perf_mode=DR,
                        )
                    if hc % 2 == 0:
                        nc.scalar.activation(
                            h2_sb[:, hc, :], h2p, AF.Relu,
                            bias=f32_sb[:, hc : hc + 1],
                        )
                    else:
                        nc.vector.tensor_scalar(
                            h2_sb[:, hc, :], h2p, f32_sb[:, hc : hc + 1], 0.0,
                            op0=ALU.add, op1=ALU.max,
                        )

                # previous tile's pooling matmuls (e-chain completed on the
                # vector engine during our h2 matmuls)
                if prev_pool is not None:
                    emit_pool(*prev_pool)
                    emit_den(prev_pool[0], prev_pool[2])

                # logits -> e = exp(logits)  (b3 dropped: cancels in softmax)
                lp = ppool.tile([1, MT], FP32, tag="lp", bufs=1)
                for kc in range(0, KC, 2):
                    nc.tensor.matmul(
                        lp,
                        w8_sb[:, kc : kc + 2, 2 * H : 2 * H + 1],
                        h2_sb[:, kc : kc + 2, :],
                        start=(kc == 0),
                        stop=(kc == KC - 2),
                        perf_mode=DR,
                    )
                e_row = wpool.tile([1, MT], FP32, tag="erow", bufs=2)
                nc.scalar.activation(e_row, lp, AF.Exp)
                prev = (j, ps_bf, e_row)

            jp, p_psbf, p_erow = prev
            stm_e = emit_echain(jp, p_erow)
            emit_pool(jp, p_psbf, stm_e)
            emit_den(jp, stm_e)

            # ---------------- finalize (all core-local) ----------------
            # collapse the 4 pool col bands: num = bcol^T @ pool_sb
            pool_sb = wpool.tile([P, H], BF16, tag="fin_poolband", bufs=1)
            nc.scalar.activation(pool_sb, pool_psum, AF.Copy)
            den_sb = wpool.tile([BSEG, 1], FP32, tag="fin_den", bufs=1)
            nc.vector.tensor_copy(den_sb, den_psum)
            rec = wpool.tile([BSEG, 1], FP32, tag="fin_rec", bufs=1)
            nc.vector.reciprocal(rec, den_sb)
            num_psum = ppool.tile([BSEG, H], FP32, tag="lp", bufs=1)
            nc.tensor.matmul(num_psum, wf_sb[:, 0, H + 2 : H + 10], pool_sb, start=True, stop=True)
            num_sb = wpool.tile([BSEG, H], FP32, tag="fin_num", bufs=1)
            nc.vector.tensor_copy(num_sb[:, 0 : H // 2], num_psum[:, 0 : H // 2])
            nc.scalar.activation(
                num_sb[:, H // 2 : H], num_psum[:, H // 2 : H], AF.Copy)
            pooled = wpool.tile([BSEG, H], FP32, tag="fin_pool", bufs=1)
            nc.vector.tensor_scalar_mul(
                pooled[:, 0 : H // 2], num_sb[:, 0 : H // 2], rec[:, 0:1])
            nc.scalar.activation(
                pooled[:, H // 2 : H], num_sb[:, H // 2 : H], AF.Copy,
                scale=rec[:, 0:1])

            identf8 = cpool.tile([BSEG, BSEG], FP32)
            make_identity(nc, identf8)
            ptp = ppool.tile([P, KC * BSEG], FP32, tag="h1h2", bufs=4)
            for kc in range(KC):
                nc.tensor.transpose(
                    ptp[:, kc * BSEG : (kc + 1) * BSEG],
                    pooled[:, kc * P : (kc + 1) * P],
                    identf8,
                )
            pooledT = wpool.tile([P, KC * BSEG], BF16, tag="fin_poolT", bufs=1)
            nc.vector.tensor_copy(pooledT, ptp)

            hf_sb = wpool.tile([P, HC * BSEG], BF16, tag="fin_hf", bufs=1)
            for hc in range(HC):
                hfp = ppool.tile([P, BSEG], FP32, tag="h1h2", bufs=4)
                for kc in range(KC):
                    nc.tensor.matmul(
                        hfp,
                        wf_sb[:, kc, hc * P : (hc + 1) * P],
                        pooledT[:, kc * BSEG : (kc + 1) * BSEG],
                        start=(kc == 0),
                        stop=(kc == KC - 1),
                    )
                if hc % 2 == 0:
                    nc.scalar.activation(
                        hf_sb[:, hc * BSEG : (hc + 1) * BSEG], hfp, AF.Relu,
                        bias=f32_sb[:, 4 + hc : 5 + hc],
                    )
                else:
                    nc.vector.tensor_scalar(
                        hf_sb[:, hc * BSEG : (hc + 1) * BSEG], hfp,
                        f32_sb[:, 4 + hc : 5 + hc], 0.0,
                        op0=ALU.add, op1=ALU.max,
                    )
            op = ppool.tile([2, BSEG], FP32, tag="eT", bufs=1)
            for hc in range(HC):
                nc.tensor.matmul(
                    op,
                    wf_sb[:, hc, H : H + 2],
                    hf_sb[:, hc * BSEG : (hc + 1) * BSEG],
                    start=(hc == 0),
                    stop=(hc == HC - 1),
                )
            o_sb = wpool.tile([2, BSEG], FP32, tag="fin_o", bufs=1)
            nc.vector.tensor_scalar_add(o_sb, op, f32_sb[0:2, 8:9])
            nc.sync.dma_start(outT, o_sb)

    nc.compile()
    return nc


def prep_in_maps(inputs):
    """Shard the full inputs into per-core input maps.  Host-side prep:
    segment-block split, zero-padding, transposes and dtype casts of the
    big tensors, one-hot materialization, seg_contrib precompute."""
    f8 = ml_dtypes.float8_e4m3
    bf = ml_dtypes.bfloat16
    ps = np.asarray(inputs["ps_data"], np.float32)
    sid = np.asarray(inputs["segment_ids"], np.int64)
    if np.any(np.diff(sid) < 0):  # tolerate unsorted ids (output invariant)
        order = np.argsort(sid, kind="stable")
        ps, sid = ps[order], sid[order]
    io_flat = np.asarray(inputs["io_embed"], np.float32).reshape(B, -1)
    W1 = np.asarray(inputs["W1"], np.float32)
    seg_full = io_flat @ W1[X:] + np.asarray(inputs["b1"], np.float32)  # (B,H)

    cnt = np.bincount(sid, minlength=B)
    blocks = cnt.reshape(NCORES, BSEG).sum(axis=1)
    tloc = int(-(-blocks.max() // MT) * MT)
    nt = tloc // MT
    offs = np.concatenate([[0], np.cumsum(blocks)]).astype(np.int64)

    def km(w):  # (512, N) -> [P, KC, N] with row c*128+p on partition p
        return np.ascontiguousarray(
            w.reshape(KC, P, -1).transpose(1, 0, 2))

    bcol = np.zeros((P, BSEG), np.float32)
    for s in range(NSUB):
        bcol[32 * s + np.arange(BSEG), np.arange(BSEG)] = 1.0
    w8 = np.zeros((P, KC, 2 * H + 16), np.float32)
    w8[:, :, 0:H] = km(W1[:X])
    w8[:, :, H : 2 * H] = km(np.asarray(inputs["W2"], np.float32))
    w8[:, :, 2 * H] = km(np.asarray(inputs["W3"], np.float32))[:, :, 0]
    wfb = np.zeros((P, KC, H + 10), np.float32)
    wfb[:, :, 0:H] = km(np.asarray(inputs["Wf1"], np.float32))
    wfb[:, :, H : H + 2] = km(np.asarray(inputs["Wf2"], np.float32))
    wfb[:, 0, H + 2 : H + 10] = bcol
    f32b = np.zeros((P, 9), np.float32)
    f32b[:, 0:HC] = np.asarray(inputs["b2"], np.float32).reshape(HC, P).T
    f32b[:, HC : 2 * HC] = np.asarray(
        inputs["bf1"], np.float32).reshape(HC, P).T
    f32b[0:2, 8] = np.asarray(inputs["bf2"], np.float32)
    shared = {
        "w8": w8.astype(f8),
        "wf": wfb.astype(bf),
        "f32": f32b,
    }
    in_maps = []
    for c in range(NCORES):
        lo, hi = offs[c], offs[c + 1]
        nl = hi - lo
        psl = np.zeros((tloc, X), np.float32)
        psl[:nl] = ps[lo:hi]
        oh = np.zeros((tloc, BSEG), np.float32)
        oh[np.arange(nl), sid[lo:hi] - c * BSEG] = 1.0
        in_maps.append(
            {
                "psm": psl.reshape(nt, NSUB, P, X).transpose(2, 0, 1, 3)
                .astype(bf),
                "pst": psl.reshape(nt, MT, KC, P).transpose(3, 0, 2, 1)
                .astype(f8),
                "stm": oh.reshape(nt, NSUB, P, BSEG).transpose(2, 0, 1, 3)
                .astype(bf),
                "segst": np.ascontiguousarray(np.concatenate(
                    [seg_full[c * BSEG : (c + 1) * BSEG], oh.T], axis=1
                )).astype(bf),
                **shared,
            }
        )
    return tloc, in_maps


_NC_CACHE = {}


def _get_nc(tloc):
    if tloc not in _NC_CACHE:
        _NC_CACHE[tloc] = build(tloc)
    return _NC_CACHE[tloc]


def run(inputs, trace=False):
    tloc, in_maps = prep_in_maps(inputs)
    nc = _get_nc(tloc)
    res = run_bass_kernel_spmd(nc, in_maps, core_ids=list(range(NCORES)), trace=trace)
    out = np.concatenate(
        [res.results[c]["outT"].T for c in range(NCORES)], axis=0
    ).astype(np.float32)
    return np.ascontiguousarray(out), res


def kernel(**inputs):
    out, _ = run(inputs)
    return out
